# revision 14
# baseline (speedup 1.0000x reference)
"""Trainium2 Bass kernel for nn_CA_85332410237583.

Computation (B=8, C=8, H=W=256, F=4):
  k = totalistic(kernels)                       # D4-symmetrized 5x5, zero mean
  z = floor(x*PV2); p = floor(conv_circ(z, k) + bias)/PV2
  h = p; 4x [h = tanh(floor(W@floor(h*PV1))/PV1)]   (per-filter 1->32->32->32->8 MLP)
  z3 = sort(h, filters)[-3]; out = clip(x + z3*update_rate, 0, 1)

Kernel strategy (one image per NeuronCore, batch-parallel over 8 cores):
  * The fixed-point quantization (floor(.*PV)/PV) perturbs values by <=1.5e-6;
    dropped.  The conv bias enters the reference at z-scale (effective size
    biases/PV2 ~ 1e-6); dropped too.
  * Layout: image rows in 16 blocks of 16; partitions hold (block, channel).
    x is staged with a circular halo, split into two row bands so the conv
    can start after the first DMA.
  * The Activation engine is the critical resource (~0.83 ns/col + ~370 ns
    fixed per instruction; tanh only runs there).  All MLP tanhs are single
    1024-wide instructions, and the chain pipeline is LAYER-SKEWED: step k
    issues L1 matmuls of chain k, L2 of chain k-1, L3 of chain k-2, then
    three tanhs of three DIFFERENT chains -- so consecutive Act instructions
    never wait on each other's psum->matmul->psum latency (~2.1 us).
  * Conv (25 accumulating fp32r matmuls per 512-px subtile, K=128, M=64)
    runs entirely up front through the same psum ring, filling p_sb.
  * L4 writes filter-major [128=(blk,c), 512] psum tiles in the x layout
    (zero-padded M=128 accumulating each quad's 32-row band); the cross-
    filter 3rd-largest runs as a running top-3 insertion (DVE, full width)
    so only two such tiles are ever live.  The s=1 half is deferred (h3
    retained in SBUF) into later steps' idle slots via a work queue.
  * The final tanh commutes with the 3rd-largest selection (monotone):
    insert network on pre-tanh values, one tanh, clip(x + update_rate*z3).
  PSUM: chain ring 3x[128,1024] (6 banks) + L4 ring 2x[128,512] (2) = 8.
"""

import os
import numpy as np

import concourse.bass as bass
import concourse.bacc as bacc
import concourse.mybir as mybir
from concourse.tile import TileContext
from concourse.bass_utils import run_bass_kernel_spmd

F32 = mybir.dt.float32
F32R = mybir.dt.float32r
AF = mybir.ActivationFunctionType
ALU = mybir.AluOpType

B, C, H, W = 8, 8, 256, 256
F = 4
RK, HALO = 5, 2
PV1 = float(np.floor(2**31 / 128))
PV2 = float(np.floor(2**31 / (RK * RK * 128)))

NBLK, RB = 16, 16          # 16 row-blocks of 16 rows
ROWS, COLS = RB + 2 * HALO, W + 2 * HALO      # 20, 260
NPIX = RB * W                                 # 4096 pixels per block
CT = 4                                        # column tiles of 1024
CTW = NPIX // CT                              # 1024
SUB = 512                                     # matmul moving-dim tile
AROWS, BROW0, BROWS = 8, 6, 14                # x band split (frame rows)
AFREE, BFREE = AROWS * COLS, BROWS * COLS     # 2080, 3640
NCH = CT * F * 4                              # 64 chains, (ct, f, q)

_cache = {}
LAST_RESULTS = None


def _totalistic(k):
    def sym(a):
        return a + np.flip(a, -2) + np.flip(a, -1) + np.flip(a, (-2, -1))
    z = 0.125 * (sym(k) + sym(np.swapaxes(k, -2, -1)))
    return z - z.mean(axis=(-2, -1), keepdims=True)


def _prep_weights(kernels, biases, W1, W2, W3, W4):
    kt = _totalistic(kernels.astype(np.float64)).astype(np.float32)  # [F,C,5,5]

    # conv lhsT: [128=(blk,c), 25*64]; col tap*64 + (f*16+blk)
    convw = np.zeros((128, 25 * 64), np.float32)
    for t in range(25):
        dy, dx = divmod(t, 5)
        for blk in range(NBLK):
            for c in range(C):
                for f in range(F):
                    convw[blk * 8 + c, t * 64 + f * 16 + blk] = kt[f, c, dy, dx]

    # L1 lhsT: [64=(f,blk), 16*128]; col (f*4+q)*128 + (b4*32+o); only the
    # 4 rows belonging to (f, q) are nonzero so rhs can be p_sb[0:64].
    l1w = np.zeros((64, 16 * 128), np.float32)
    for f in range(F):
        for q in range(4):
            for b4 in range(4):
                l1w[f * 16 + q * 4 + b4,
                    (f * 4 + q) * 128 + b4 * 32:(f * 4 + q) * 128 + b4 * 32 + 32] = W1[f, :, 0]

    # L2/L3 lhsT: [128=(b4,h), 4*128=(f,(b4,o))] block-diagonal over b4
    def bd(Wm):
        out = np.zeros((128, F * 128), np.float32)
        for f in range(F):
            for b4 in range(4):
                out[b4 * 32:b4 * 32 + 32, f * 128 + b4 * 32:f * 128 + b4 * 32 + 32] = Wm[f].T
        return out
    l2w, l3w = bd(W2), bd(W3)

    # L4 lhsT: per (q, f) a zero-padded [128, 128] block; matmul (q, f) writes
    # the full M=128=(blk,c) range with only the quad's 32-row band nonzero,
    # so the four quads of filter f accumulate into one x-layout psum tile
    # T_f[(q*32 + b4*8 + c), :].
    l4w = np.zeros((128, 16 * 128), np.float32)
    for q in range(4):
        for f in range(F):
            base = (q * 4 + f) * 128
            for b4 in range(4):
                for cc in range(C):
                    l4w[b4 * 32:b4 * 32 + 32,
                        base + q * 32 + b4 * 8 + cc] = W4[f][cc, :]
    return convw, l1w, l2w, l3w, l4w


def _stage_x(xb):
    """xb: [C, H, W] -> [128=(blk,c), AFREE+BFREE]: circular-halo frame rows
    0..8 then rows 6..20 (overlap keeps every conv tap and final read within
    a single band)."""
    frame = np.empty((128, ROWS, COLS), np.float32)
    rows = (np.arange(-HALO, RB + HALO)[None, :] + np.arange(NBLK)[:, None] * RB) % H
    cols = np.arange(-HALO, W + HALO) % W
    for blk in range(NBLK):
        frame[blk * 8:blk * 8 + 8] = xb[:, rows[blk]][:, :, cols]
    out = np.concatenate(
        [frame[:, :AROWS].reshape(128, AFREE),
         frame[:, BROW0:BROW0 + BROWS].reshape(128, BFREE)], axis=1)
    return np.ascontiguousarray(out)


def _build_nc(update_rate):
    nc = bacc.Bacc(trn_type="TRN2")

    xd = nc.dram_tensor("xsb", [128, AFREE + BFREE], F32R, kind="ExternalInput")
    cwd = nc.dram_tensor("convw", [128, 1600], F32R, kind="ExternalInput")
    w1d = nc.dram_tensor("l1w", [64, 16 * 128], F32R, kind="ExternalInput")
    w2d = nc.dram_tensor("l2w", [128, F * 128], F32R, kind="ExternalInput")
    w3d = nc.dram_tensor("l3w", [128, F * 128], F32R, kind="ExternalInput")
    w4d = nc.dram_tensor("l4w", [128, 16 * 128], F32R, kind="ExternalInput")
    outd = nc.dram_tensor("out", [128, NPIX], F32, kind="ExternalOutput")

    ur = float(update_rate)

    with TileContext(nc) as tc:
        with (
            tc.tile_pool(name="w", bufs=1) as wp,
            tc.tile_pool(name="sb", bufs=2) as sp,
            tc.tile_pool(name="h12", bufs=4) as h12p,
            tc.tile_pool(name="h3", bufs=8) as h3p,
            tc.tile_pool(name="psc", bufs=3, space="PSUM") as cp,
            tc.tile_pool(name="pst", bufs=2, space="PSUM") as tp,
        ):
            xa = wp.tile([128, AFREE], F32R, tag="xa")
            xbt = wp.tile([128, BFREE], F32R, tag="xb")
            cw = wp.tile([128, 1600], F32R, tag="cw")
            w1 = wp.tile([64, 16 * 128], F32R, tag="w1")
            w2 = wp.tile([128, F * 128], F32R, tag="w2")
            w3 = wp.tile([128, F * 128], F32R, tag="w3")
            w4 = wp.tile([128, 16 * 128], F32R, tag="w4")
            p_sb = wp.tile([64, NPIX], F32R, tag="p")
            out_sb = wp.tile([128, NPIX], F32, tag="o")

            nc.sync.dma_start(out=xa[:], in_=xd[:, 0:AFREE])
            nc.sync.dma_start(out=cw[:], in_=cwd[:])
            nc.sync.dma_start(out=xbt[:], in_=xd[:, AFREE:AFREE + BFREE])
            nc.sync.dma_start(out=w1[:], in_=w1d[:])
            nc.sync.dma_start(out=w2[:], in_=w2d[:])
            nc.sync.dma_start(out=w3[:], in_=w3d[:])
            nc.sync.dma_start(out=w4[:], in_=w4d[:])

            xra = xa[:].rearrange("p (r c) -> p r c", c=COLS)   # rows 0..8
            xrb = xbt[:].rearrange("p (r c) -> p r c", c=COLS)  # rows 6..20

            # ---- phase 1: conv for every column tile, through the psum ring
            for ct in range(CT):
                acc = cp.tile([128, CTW], F32, tag="c", name=f"conv_{ct}")
                for t in range(25):
                    dy, dx = divmod(t, 5)
                    for s in range(2):
                        r0 = 4 * ct + 2 * s + dy
                        if r0 >= BROW0:
                            rhs = xrb[:, r0 - BROW0:r0 - BROW0 + 2, dx:dx + W]
                        else:
                            rhs = xra[:, r0:r0 + 2, dx:dx + W]
                        outap = acc[0:64, s * SUB:(s + 1) * SUB].rearrange(
                            "p (a b) -> p a b", b=W)
                        nc.tensor.matmul(
                            outap, lhsT=cw[:, t * 64:t * 64 + 64], rhs=rhs,
                            start=(t == 0), stop=(t == 24))
                nc.vector.tensor_copy(
                    p_sb[:, ct * CTW:(ct + 1) * CTW], acc[0:64, :])

            # ---- phase 2: layer-skewed chain pipeline over 64 chains ----
            # chain j = (ct=j//16, f=(j//4)%4, q=j%4)
            wq = []        # filler work: sort inserts, deferred s=1 L4s, out
            mreg = {}      # (s,) running top-3 tiles for current ct
            h1t = [None] * NCH
            h2t = [None] * NCH
            h3t = [None] * NCH
            tf0 = {}       # f -> live s=0 psum tile
            tf1 = {}

            def pop_work(n):
                for _ in range(n):
                    if wq:
                        wq.pop(0)()

            def insert_ops(fi, T, s, ct):
                """Running top-3 insert of T (=[128,SUB] psum) for half s."""
                if fi == 0:
                    for m in range(3):
                        mreg[(ct, s, m)] = sp.tile(
                            [128, SUB], F32, tag=f"m{m}{s}",
                            name=f"m{m}_{ct}_{s}")
                m1, m2, m3 = (mreg[(ct, s, m)] for m in range(3))
                if fi == 0:
                    nc.vector.tensor_copy(m1[:], T[:])
                elif fi == 1:
                    nc.vector.tensor_tensor(m2[:], m1[:], T[:], ALU.min)
                    nc.vector.tensor_tensor(m1[:], m1[:], T[:], ALU.max)
                elif fi == 2:
                    lo = sp.tile([128, SUB], F32, tag="tt",
                                 name=f"tt_{ct}_{s}")
                    nc.vector.tensor_tensor(lo[:], m1[:], T[:], ALU.min)
                    nc.vector.tensor_tensor(m3[:], m2[:], lo[:], ALU.min)
                    nc.vector.tensor_tensor(m2[:], m2[:], lo[:], ALU.max)
                else:
                    # z3 = 3rd largest = max(m3, min(m2, T)); tanh; output
                    z3 = sp.tile([128, SUB], F32, tag=f"z3{s}",
                                 name=f"z3_{ct}_{s}")
                    nc.vector.tensor_tensor(z3[:], m2[:], T[:], ALU.min)
                    nc.vector.tensor_tensor(z3[:], m3[:], z3[:], ALU.max)
                    nc.scalar.activation(z3[:], z3[:], AF.Tanh)
                    if ur != 1.0:
                        nc.vector.tensor_scalar_mul(z3[:], z3[:], ur)
                    r = 4 * ct + 2 * s
                    if ct == 0:
                        xv = xra[:, HALO + r:HALO + r + 2, HALO:HALO + W]
                    else:
                        xv = xrb[:, HALO + r - BROW0:HALO + r - BROW0 + 2,
                                 HALO:HALO + W]
                    ocols = slice(ct * CTW + s * SUB, ct * CTW + (s + 1) * SUB)
                    ov = out_sb[:, ocols].rearrange("p (a b) -> p a b", b=W)
                    nc.vector.tensor_tensor(
                        ov, xv.bitcast(F32),
                        z3[:].rearrange("p (a b) -> p a b", b=W), ALU.add)
                    nc.vector.tensor_scalar(
                        out_sb[:, ocols], out_sb[:, ocols],
                        0.0, 1.0, ALU.max, ALU.min)
                    if s == 1:
                        nc.sync.dma_start(
                            out=outd[:, ct * CTW:(ct + 1) * CTW],
                            in_=out_sb[:, ct * CTW:(ct + 1) * CTW])

            def epilogue(f, fi, ct):
                """Queued after L4 s=0 of (f, q=3): insert s=0, then the
                deferred s=1 L4 matmuls, then insert s=1 (+ output on f=3)."""
                base = (ct * F + f) * 4

                def ins0():
                    insert_ops(fi, tf0[f], 0, ct)
                    tf1[f] = tp.tile([128, SUB], F32, tag="t",
                                     name=f"tf1_{ct}_{f}")
                yield ins0
                for q in range(4):
                    def s1mm(q=q, f=f):
                        nc.tensor.matmul(
                            tf1[f][:, :],
                            lhsT=w4[:, (q * 4 + f) * 128:(q * 4 + f + 1) * 128],
                            rhs=h3t[base + q][:, SUB:2 * SUB],
                            start=(q == 0), stop=(q == 3))
                    yield s1mm

                def ins1():
                    insert_ops(fi, tf1[f], 1, ct)
                yield ins1

            for step in range(NCH + 3):
                pop_work(2)
                if step < NCH:
                    j, ct = step, step // 16
                    f, q = (j // 4) % 4, j % 4
                    ch1 = cp.tile([128, CTW], F32, tag="c", name=f"ch1_{j}")
                    for s in range(2):
                        nc.tensor.matmul(
                            ch1[:, s * SUB:(s + 1) * SUB],
                            lhsT=w1[:, (f * 4 + q) * 128:(f * 4 + q + 1) * 128],
                            rhs=p_sb[0:64, ct * CTW + s * SUB:
                                     ct * CTW + (s + 1) * SUB],
                            start=True, stop=True)
                    ch1_j = ch1
                if 1 <= step < NCH + 1:
                    j = step - 1
                    ct, f = j // 16, (j // 4) % 4
                    ch2 = cp.tile([128, CTW], F32, tag="c", name=f"ch2_{j}")
                    for s in range(2):
                        cs = slice(s * SUB, (s + 1) * SUB)
                        nc.tensor.matmul(
                            ch2[:, cs], lhsT=w2[:, f * 128:(f + 1) * 128],
                            rhs=h1t[j][:, cs], start=True, stop=True)
                    ch2_j = ch2
                if 2 <= step < NCH + 2:
                    j = step - 2
                    ct, f = j // 16, (j // 4) % 4
                    ch3 = cp.tile([128, CTW], F32, tag="c", name=f"ch3_{j}")
                    for s in range(2):
                        cs = slice(s * SUB, (s + 1) * SUB)
                        nc.tensor.matmul(
                            ch3[:, cs], lhsT=w3[:, f * 128:(f + 1) * 128],
                            rhs=h2t[j][:, cs], start=True, stop=True)
                    ch3_j = ch3

                # three tanhs of three different chains, back to back
                if step < NCH:
                    j = step
                    h1t[j] = h12p.tile([128, CTW], F32R, tag="h12",
                                       name=f"h1_{j}")
                    nc.scalar.activation(h1t[j][:], ch1_j[:], AF.Tanh)
                if 1 <= step < NCH + 1:
                    j = step - 1
                    h2t[j] = h12p.tile([128, CTW], F32R, tag="h12",
                                       name=f"h2_{j}")
                    nc.scalar.activation(h2t[j][:], ch2_j[:], AF.Tanh)
                if 2 <= step < NCH + 2:
                    j = step - 2
                    h3t[j] = h3p.tile([128, CTW], F32R, tag="h3",
                                      name=f"h3_{j}")
                    nc.scalar.activation(h3t[j][:], ch3_j[:], AF.Tanh)

                pop_work(2)
                if 3 <= step:
                    j = step - 3
                    ct, f, q = j // 16, (j // 4) % 4, j % 4
                    if q == 0:
                        tf0[f] = tp.tile([128, SUB], F32, tag="t",
                                         name=f"tf0_{ct}_{f}")
                    nc.tensor.matmul(
                        tf0[f][:, :],
                        lhsT=w4[:, (q * 4 + f) * 128:(q * 4 + f + 1) * 128],
                        rhs=h3t[j][:, 0:SUB],
                        start=(q == 0), stop=(q == 3))
                    if q == 3:
                        wq.extend(epilogue(f, f, ct))

            while wq:
                pop_work(1)
    nc.finalize()
    return nc


def kernel(x, kernels, biases, W1, W2, W3, W4, update_rate):
    global LAST_RESULTS
    x = np.ascontiguousarray(np.asarray(x, dtype=np.float32))
    kernels = np.asarray(kernels, dtype=np.float32)
    biases = np.asarray(biases, dtype=np.float32)
    W1 = np.asarray(W1, dtype=np.float32)
    W2 = np.asarray(W2, dtype=np.float32)
    W3 = np.asarray(W3, dtype=np.float32)
    W4 = np.asarray(W4, dtype=np.float32)
    ur = float(np.asarray(update_rate))

    key = ("nc", ur)
    if key not in _cache:
        _cache[key] = _build_nc(ur)
    nc = _cache[key]

    convw, l1w, l2w, l3w, l4w = _prep_weights(
        kernels, biases, W1, W2, W3, W4)
    shared = {
        "convw": np.ascontiguousarray(convw),
        "l1w": np.ascontiguousarray(l1w),
        "l2w": np.ascontiguousarray(l2w),
        "l3w": np.ascontiguousarray(l3w),
        "l4w": np.ascontiguousarray(l4w),
    }
    in_maps = []
    for b in range(B):
        m = dict(shared)
        m["xsb"] = _stage_x(x[b])
        in_maps.append(m)

    trace = bool(int(os.environ.get("KERNEL_TRACE", "0")))
    res = run_bass_kernel_spmd(nc, in_maps, list(range(B)), trace=trace)
    LAST_RESULTS = res

    out = np.empty((B, C, H, W), np.float32)
    for b in range(B):
        ob = res.results[b]["out"].reshape(NBLK, C, RB, W)
        out[b] = ob.transpose(1, 0, 2, 3).reshape(C, H, W)
    return out
